# revision 15
# baseline (speedup 1.0000x reference)
"""4-layer GATv2 forward pass on 8 TRN2 NeuronCores (Bass/Tile).

Strategy (node/dst partitioning, pipelined cross-core xl exchange):
  - Nodes are padded to 20480 and split into 8 contiguous slices of 2560
    (20 blocks of 128 dst nodes per core).  Each core owns the segment
    softmax + weighted scatter for its dst nodes, so all softmax
    reductions are core-local.
  - Edges (with self loops appended) are routed to the core/block that
    owns their dst.  Per (core, block) edge counts are padded to a
    shared multiple of 128 (G[b] groups of 128 edges) so one NEFF works
    for all 8 cores.
  - The per-layer xl tables (gather source for xl[src]) are produced
    INSIDE the previous layer's edge phase: as each dst block's h is
    computed, it is DMA-transposed (XBAR) and immediately matmul'ed by
    the next layer's wl/wr.  xl rows stream to a local DRAM buffer that
    is AllGather'ed in 4 chunks overlapping the edge phase, so the
    inter-layer collective is almost entirely hidden.  Layer 1 needs no
    collective at all: x^T is staged as a replicated input and every
    core computes the full layer-1 xl table itself.
  - Per-edge work is edge-major (partition = edge % 128): dma_gather of
    xl[src] rows, z = Sel^T @ xr + I @ xl (PE), leaky-relu (scalar),
    per-head dot with `a` (DVE reduce), exp, then segment softmax
    denominator and the alpha-weighted sum of xl[src] as one PE matmul
    per 128-edge group against a one-hot dst selector (SelT).
  - Softmax uses exp(logit) directly (no running max): logits are O(10)
    here, fp32 exp is exact enough, and the math is identical to the
    reference's shifted softmax.

kernel(**inputs) takes the full problem inputs and returns the full
[20000, 16] fp32 output.
"""

import numpy as np

import concourse.bass as bass
import concourse.bacc as bacc
import concourse.mybir as mybir
import concourse.tile as tile
from concourse.bass_utils import run_bass_kernel_spmd
from concourse.masks import make_identity

F16 = mybir.dt.float16
BF16 = mybir.dt.bfloat16
F32 = mybir.dt.float32
I16 = mybir.dt.int16
P = 128

# model dims (fixed by the problem)
N_REAL = 20000
E_RAW = 320000
IN_CH = 128
HID = 64
HEADS = 4
OUT_CH = 16
SLOPE = 0.2

MASK_NEG = -50.0  # additive logit bias for pad edges
DEN_EPS = 1e-12   # keeps reciprocal() in range for edgeless (pad) dst rows

N_CORES = 8
NPC = 2560          # nodes per core (padded)
NBLK = NPC // P     # 20 dst blocks per core
NPAD = N_CORES * NPC
NCHUNK = 4          # xl AllGather chunks per layer
CBLK = NBLK // NCHUNK           # 5 blocks per chunk
CROWS = CBLK * P                # 640 rows per core per chunk
# emit AllGather for chunk k after this edge block:
AG_EMIT = (11, 14, 17, 19)

# per-layer table widths (c_tbl) and heads
LAYERS = [
    dict(c_in=IN_CH, c_tbl=HEADS * HID, n_h=HEADS, c_h=HID),
    dict(c_in=HEADS * HID, c_tbl=HEADS * HID, n_h=HEADS, c_h=HID),
    dict(c_in=HEADS * HID, c_tbl=HEADS * HID, n_h=HEADS, c_h=HID),
    dict(c_in=HEADS * HID, c_tbl=P, n_h=1, c_h=P),  # 16 real, padded to 128
]
C_TBL_MAX = max(L["c_tbl"] for L in LAYERS)


def row_of_node(n):
    """xl-table row layout shared by all 4 layers.  With one whole-table
    AllGather per layer the table is plain node-major (row == node id).
    (A chunked-AllGather layout would be
    ((b//CBLK)*N_CORES + c)*CBLK*P + (b%CBLK)*P + p.)"""
    return n


# ---------------------------------------------------------------------------
# host-side graph preprocessing
# ---------------------------------------------------------------------------

def prep_graph(edge_index):
    """Route edges (plus self loops) to (core, block) by dst; build per-core
    gather-index / dst-local / mask arrays in the exact SBUF layouts the
    kernel consumes."""
    n = N_REAL
    src = np.concatenate([np.asarray(edge_index[0], np.int64),
                          np.arange(n, dtype=np.int64)])
    dst = np.concatenate([np.asarray(edge_index[1], np.int64),
                          np.arange(n, dtype=np.int64)])
    assert src.min() >= 0 and src.max() < n and dst.min() >= 0 and dst.max() < n

    gblk = dst // P                       # global block id (core-major)
    order = np.argsort(gblk, kind="stable")
    src, dst, gblk = src[order], dst[order], gblk[order]

    nblk_tot = N_CORES * NBLK
    counts = np.bincount(gblk, minlength=nblk_tot).reshape(N_CORES, NBLK)
    G = np.maximum(1, (counts.max(axis=0) + P - 1) // P).astype(np.int64)  # [NBLK]
    W = int(G.sum())

    # split edges per (core, block)
    starts = np.zeros(nblk_tot + 1, np.int64)
    np.cumsum(counts.reshape(-1), out=starts[1:])

    # node id -> chunked xl-table row (same for every layer)
    rows = row_of_node(np.arange(n, dtype=np.int64))

    import ml_dtypes
    per_core = []
    for c in range(N_CORES):
        xl_idx = np.zeros((P, 8 * W), np.int16)
        mbias = np.full((P, W), MASK_NEG, np.float16)
        # per block: [sel (gG*128) | selt (gG*128)] back to back
        selcat = np.zeros((P, 2 * W * P), ml_dtypes.bfloat16)
        off = 0
        for b in range(NBLK):
            gb = c * NBLK + b
            s, e = starts[gb], starts[gb + 1]
            nreal = int(e - s)
            g = int(G[b])
            npad_e = g * P
            fsrc = np.zeros(npad_e, np.int64)
            fdl = np.zeros(npad_e, np.int64)
            fm = np.full(npad_e, MASK_NEG, np.float32)
            fsrc[:nreal] = rows[src[s:e]]
            fdl[:nreal] = dst[s:e] % P
            fm[:nreal] = 0.0
            # edge i -> partition i % 128, group i // 128
            mbias[:, off:off + g] = fm.reshape(g, P).T.astype(np.float16)
            fdl_blk = fdl.reshape(g, P)
            co = 2 * off * P
            # Sel[d, g*128+j] = (fdl[g*128+j] == d)  [xr expansion matmul lhsT]
            selcat[:, co:co + g * P] = (
                fdl_blk[None, :, :] == np.arange(P)[:, None, None]
            ).reshape(P, g * P).astype(ml_dtypes.bfloat16)
            # SelT[p, g*128+d] = (fdl[g*128+p] == d)  [segment-sum matmul lhsT]
            selcat[:, co + g * P:co + 2 * g * P] = (
                fdl_blk[:, :, None] == np.arange(P)[None, None, :]
            ).transpose(1, 0, 2).reshape(P, g * P).astype(ml_dtypes.bfloat16)
            # wrapped idx layout: wrapped[p, s] = flat[s*16 + p], replicated
            # into all 8 16-partition groups (one per GPSIMD Q7 core)
            xl_idx[:, 8 * off:8 * (off + g)] = np.tile(
                fsrc.astype(np.int16).reshape(-1, 16).T, (8, 1))
            off += g
        per_core.append(dict(xl_idx=xl_idx, mbias=mbias, selcat=selcat))
    return [int(g) for g in G], per_core


# ---------------------------------------------------------------------------
# bass program
# ---------------------------------------------------------------------------

def build_nc(G, debug=False):
    """Build the (single, SPMD) bass program."""
    nl = len(LAYERS)
    W = sum(G)
    Gmax = max(G)

    nc = bacc.Bacc("TRN2", target_bir_lowering=False, debug=False,
                   num_devices=N_CORES, num_swdge_queues=4)

    # replicated full x^T in the chunked row layout; own-slice x^T for xr1
    xT_d = nc.dram_tensor("xT", [P, NPAD], BF16, kind="ExternalInput")
    xTo_d = nc.dram_tensor("xTo", [P, NPC], BF16, kind="ExternalInput")
    xl_idx_d = nc.dram_tensor("xl_idx", [P, 8 * W], I16, kind="ExternalInput")
    selcat_d = nc.dram_tensor("selcat", [P, 2 * W * P], BF16,
                              kind="ExternalInput")
    mbias_d = nc.dram_tensor("mbias", [P, W], F16, kind="ExternalInput")
    w_d, a_d = [], []
    for l, L in enumerate(LAYERS):
        wl = nc.dram_tensor(f"w{l}l", [L["c_in"], L["c_tbl"]], BF16,
                            kind="ExternalInput")
        wr = nc.dram_tensor(f"w{l}r", [L["c_in"], L["c_tbl"]], BF16,
                            kind="ExternalInput")
        w_d.append((wl, wr))
        a_d.append(nc.dram_tensor(f"a{l}", [P, L["c_tbl"]], F16,
                                  kind="ExternalInput"))
    out_d = nc.dram_tensor("out", [NPC, OUT_CH], F32, kind="ExternalOutput")
    if debug:
        dbg_tbl0 = nc.dram_tensor("dbg_tbl0", [NPAD, 256], BF16,
                                  kind="ExternalOutput")
        dbg_tbl1 = nc.dram_tensor("dbg_tbl1", [NPAD, 256], BF16,
                                  kind="ExternalOutput")
        dbg_loc1 = nc.dram_tensor("dbg_loc1", [NPC, 256], BF16,
                                  kind="ExternalOutput")
        dbg_xr1 = nc.dram_tensor("dbg_xr1", [P, NBLK * 256], BF16,
                                 kind="ExternalOutput")
        dbg_h1 = nc.dram_tensor("dbg_h1", [NPC, 256], BF16,
                                kind="ExternalOutput")
        dbg_h1T = nc.dram_tensor("dbg_h1T", [P, NBLK * 256], BF16,
                                 kind="ExternalOutput")
        dbg_xlg = nc.dram_tensor("dbg_xlg", [P, 18 * 256], BF16,
                                 kind="ExternalOutput")
        dbg_lrz = nc.dram_tensor("dbg_lrz", [P, 18 * 256], F16,
                                 kind="ExternalOutput")
        dbg_ex = nc.dram_tensor("dbg_ex", [P, 18 * HEADS], BF16,
                                kind="ExternalOutput")

    rg = [list(range(N_CORES))]

    with tile.TileContext(nc) as tc:
        with (
            tc.tile_pool(name="const", bufs=1) as cpool,
            tc.tile_pool(name="wts", bufs=2) as wpool,
            tc.tile_pool(name="mm", bufs=4) as mpool,
            tc.tile_pool(name="gath", bufs=2) as gpool,
            tc.tile_pool(name="edge", bufs=2) as epool,
            tc.tile_pool(name="small", bufs=2) as spool,
            tc.tile_pool(name="psum", bufs=2, space="PSUM") as ppool,
            tc.tile_pool(name="dram", bufs=1, space="DRAM") as dpool,
        ):
            # ---- persistent constants -------------------------------------
            ident = cpool.tile([P, P], BF16, tag="ident")
            make_identity(nc, ident[:])
            mb_sb = cpool.tile([P, W], F16, tag="mbias")
            nc.sync.dma_start(out=mb_sb[:], in_=mbias_d[:])

            # ---- per-layer DRAM scratch -----------------------------------
            # xl gather tables (chunked row layout). L1 local (computed
            # replicated), L2-4 shared (AllGather outputs).
            xl_tbl = []
            xl_loc = [None]
            for l, L in enumerate(LAYERS):
                xl_tbl.append(dpool.tile(
                    [NPAD, L["c_tbl"]], BF16, tag=f"xltbl{l}",
                    name=f"xltbl{l}",
                    addr_space="Local" if l == 0 else "Shared"))
                if l > 0:
                    xl_loc.append(dpool.tile([NPC, L["c_tbl"]], BF16,
                                             tag=f"xlloc{l}", name=f"xlloc{l}"))

            # ---- L1 node phase (replicated; no collective) ----------------
            wl_sb = wpool.tile([P, 2 * C_TBL_MAX], BF16, tag="wl")
            wr_sb = wpool.tile([P, 2 * C_TBL_MAX], BF16, tag="wr")
            C1 = LAYERS[0]["c_tbl"]
            nc.sync.dma_start(out=wl_sb[:, :C1], in_=w_d[0][0][:, :])
            nc.sync.dma_start(out=wr_sb[:, :C1], in_=w_d[0][1][:, :])
            for rb in range(NPAD // P):
                xt = mpool.tile([P, P], BF16, tag="xt")
                nc.sync.dma_start(out=xt[:], in_=xT_d[:, rb * P:(rb + 1) * P])
                ps1 = ppool.tile([P, C1], F32, tag="ps_t")
                nc.tensor.matmul(ps1[:], lhsT=xt[:], rhs=wl_sb[:, :C1],
                                 start=True, stop=True)
                xl1t = mpool.tile([P, C1], BF16, tag="xl1t")
                if rb % 2 == 0:
                    nc.scalar.activation(xl1t[:], ps1[:],
                                         mybir.ActivationFunctionType.Copy)
                else:
                    nc.vector.tensor_copy(out=xl1t[:], in_=ps1[:])
                nc.sync.dma_start(out=xl_tbl[0][rb * P:(rb + 1) * P, :],
                                  in_=xl1t[:])
            xr_cur = wpool.tile([P, NBLK * C_TBL_MAX], BF16, tag="xr_sb")
            for b in range(NBLK):
                xto = mpool.tile([P, P], BF16, tag="xt")
                nc.sync.dma_start(out=xto[:], in_=xTo_d[:, b * P:(b + 1) * P])
                ps1 = ppool.tile([P, C1], F32, tag="ps_t")
                nc.tensor.matmul(ps1[:], lhsT=xto[:], rhs=wr_sb[:, :C1],
                                 start=True, stop=True)
                if b % 2 == 0:
                    nc.scalar.activation(xr_cur[:, b * C1:(b + 1) * C1], ps1[:],
                                         mybir.ActivationFunctionType.Copy)
                else:
                    nc.vector.tensor_copy(out=xr_cur[:, b * C1:(b + 1) * C1],
                                          in_=ps1[:])

            # ---- layers ----------------------------------------------------
            for l, L in enumerate(LAYERS):
                C, n_h, c_h = L["c_tbl"], L["n_h"], L["c_h"]
                EC = C + n_h
                last = l + 1 == nl
                if not last:
                    Cn = LAYERS[l + 1]["c_tbl"]
                    # next layer's weights, packed [wl_kc | wr_kc] per kc so
                    # the tail is one accumulation chain per PSUM bank
                    wlr_nxt = wpool.tile([P, 4 * C_TBL_MAX], BF16, tag="wlr")
                    for kc in range(2):
                        nc.sync.dma_start(
                            out=wlr_nxt[:, kc * 2 * Cn:kc * 2 * Cn + Cn],
                            in_=w_d[l + 1][0][kc * P:(kc + 1) * P, :])
                        nc.sync.dma_start(
                            out=wlr_nxt[:, kc * 2 * Cn + Cn:(kc + 1) * 2 * Cn],
                            in_=w_d[l + 1][1][kc * P:(kc + 1) * P, :])
                    xr_nxt = wpool.tile([P, NBLK * C_TBL_MAX], BF16,
                                        tag="xr_sb")
                a_rep = wpool.tile([P, Gmax * C_TBL_MAX], F16, tag="arep")
                nc.sync.dma_start(
                    out=a_rep[:, :Gmax * C].rearrange("p (g c) -> p g c",
                                                      g=Gmax),
                    in_=a_d[l][:].rearrange("p (g c) -> p g c", g=1)
                        .to_broadcast([P, Gmax, C]))

                # ---- edge phase -------------------------------------------
                qn = [0]

                def gather_rows(tbl_ap, out_tile, off_g, n_g, C_, nm):
                    """Gather n_g*128 rows from tbl_ap into out_tile
                    [P, n_g, C_], in <=1024-idx chunks (ucode ring limit)
                    round-robined over the 4 SWDGE queues."""
                    for k0 in range(0, n_g, 8):
                        gk = min(8, n_g - k0)
                        it = gpool.tile([P, 8 * 8], I16, tag="idxt",
                                        name=f"idxt_{nm}_{k0}", bufs=8)
                        nc.sync.dma_start(
                            out=it[:, :8 * gk],
                            in_=xl_idx_d[:, 8 * (off_g + k0):
                                         8 * (off_g + k0 + gk)])
                        nc.gpsimd.dma_gather(
                            out_ap=out_tile[:, k0 * C_:(k0 + gk) * C_]
                                .rearrange("p (g c) -> p g c", c=C_),
                            in_ap=tbl_ap,
                            idxs_ap=it[:, :8 * gk],
                            num_idxs=gk * P, num_idxs_reg=gk * P,
                            elem_size=C_, queue_num=qn[0] % 4)
                        qn[0] += 1

                for b in range(NBLK):
                    gG = G[b]
                    off = sum(G[:b])
                    xl_g = gpool.tile([P, Gmax * C_TBL_MAX], BF16, tag="xl_g",
                                      bufs=4)
                    gather_rows(xl_tbl[l][:, :], xl_g, off, gG, C,
                                f"xl{l}_{b}")
                    xlb = xl_g[:, :gG * C]
                    # [sel | selt] for this block in one DMA
                    selcat = epool.tile([P, 2 * Gmax * P], BF16, tag="selcat")
                    nc.sync.dma_start(
                        out=selcat[:, :2 * gG * P],
                        in_=selcat_d[:, 2 * off * P:2 * (off + gG) * P])
                    sel = selcat[:, :gG * P]          # [d, e]-major
                    selt = selcat[:, gG * P:2 * gG * P]  # [e, d]-major
                    # z (per pair of groups) in PSUM:
                    #   z_g = Sel_g^T @ xr_blk + I^T @ xl_g   -> leaky relu
                    # NOTE: start=True wipes the whole PSUM bank, so each
                    # bank's accumulation must begin with ONE start=True
                    # matmul covering every region used (the ident batch),
                    # then accumulate into sub-regions with start=False.
                    lrz = epool.tile([P, Gmax * C_TBL_MAX], F16, tag="lrz")
                    for g0 in range(0, gG, 2):
                        gns = min(2, gG - g0)
                        ps_z = ppool.tile([P, 2 * C_TBL_MAX], F32, tag="ps_z")
                        nc.tensor.matmul(
                            ps_z[:, :gns * C], lhsT=ident[:],
                            rhs=xl_g[:, g0 * C:(g0 + gns) * C],
                            start=True, stop=False)
                        for gg in range(g0, g0 + gns):
                            sl = slice((gg - g0) * C, (gg - g0 + 1) * C)
                            nc.tensor.matmul(
                                ps_z[:, sl], lhsT=sel[:, gg * P:(gg + 1) * P],
                                rhs=xr_cur[:, b * C:(b + 1) * C],
                                start=False, stop=(gg == g0 + gns - 1))
                        nc.scalar.activation(
                            lrz[:, g0 * C:(g0 + gns) * C],
                            ps_z[:, :gns * C],
                            mybir.ActivationFunctionType.Prelu,
                            alpha=SLOPE)
                    # a * LR(z)
                    alr = epool.tile([P, Gmax * C_TBL_MAX], F16, tag="alr")
                    nc.vector.tensor_tensor(out=alr[:, :gG * C],
                                            in0=lrz[:, :gG * C],
                                            in1=a_rep[:, :gG * C],
                                            op=mybir.AluOpType.mult)
                    # logits: single reduce over c_h
                    logits = spool.tile([P, Gmax * HEADS], F32, tag="logits")
                    nc.vector.tensor_reduce(
                        out=logits[:, :gG * n_h],
                        in_=alr[:, :gG * C].rearrange(
                            "p (gh c) -> p gh c", c=c_h),
                        axis=mybir.AxisListType.X, op=mybir.AluOpType.add)
                    # pad-edge mask as additive bias
                    logm = spool.tile([P, Gmax * HEADS], F32, tag="logm")
                    nc.vector.tensor_tensor(
                        out=logm[:, :gG * n_h].rearrange(
                            "p (g h) -> p g h", h=n_h),
                        in0=logits[:, :gG * n_h].rearrange(
                            "p (g h) -> p g h", h=n_h),
                        in1=mb_sb[:, off:off + gG]
                            .rearrange("p (g h) -> p g h", h=1)
                            .to_broadcast([P, gG, n_h]),
                        op=mybir.AluOpType.add)
                    ex = spool.tile([P, Gmax * HEADS], BF16, tag="ex")
                    nc.scalar.activation(ex[:, :gG * n_h], logm[:, :gG * n_h],
                                         mybir.ActivationFunctionType.Exp)
                    if debug and l == 0 and b == 0:
                        nc.sync.dma_start(out=dbg_xlg[:, :gG * C],
                                          in_=xl_g[:, :gG * C])
                        nc.sync.dma_start(out=dbg_lrz[:, :gG * C],
                                          in_=lrz[:, :gG * C])
                        nc.sync.dma_start(out=dbg_ex[:, :gG * n_h],
                                          in_=ex[:, :gG * n_h])
                    # edata = [ex * xl[src] | ex]   (ex broadcast fused on DVE)
                    edata = epool.tile([P, Gmax * (C_TBL_MAX + HEADS)], BF16,
                                       tag="edata")
                    ed3 = edata[:, :gG * EC].rearrange("p (g c) -> p g c", c=EC)
                    nc.vector.tensor_tensor(
                        out=ed3[:, :, :C].rearrange(
                            "p g (h ch) -> p g h ch", ch=c_h),
                        in0=xlb.rearrange("p (g h ch) -> p g h ch",
                                          g=gG, h=n_h),
                        in1=ex[:, :gG * n_h].rearrange(
                            "p (g h ch) -> p g h ch", h=n_h, ch=1)
                            .to_broadcast([P, gG, n_h, c_h]),
                        op=mybir.AluOpType.mult)
                    nc.vector.tensor_copy(
                        out=ed3[:, :, C:],
                        in_=ex[:, :gG * n_h].rearrange("p (g h) -> p g h",
                                                       h=n_h))
                    # segment sums via PE: psum[d, :] += SelT_g^T @ edata_g
                    ps_nd = ppool.tile([P, EC], F32, tag="ps_nd")
                    for g in range(gG):
                        nc.tensor.matmul(
                            ps_nd[:], lhsT=selt[:, g * P:(g + 1) * P],
                            rhs=edata[:, g * EC:(g + 1) * EC],
                            start=(g == 0), stop=(g == gG - 1))
                    den_e = spool.tile([P, HEADS], F32, tag="den_e")
                    nc.vector.tensor_scalar(
                        out=den_e[:, :n_h], in0=ps_nd[:, C:], scalar1=DEN_EPS,
                        scalar2=None, op0=mybir.AluOpType.add)
                    rden = spool.tile([P, HEADS], F32, tag="rden")
                    nc.vector.reciprocal(rden[:, :n_h], den_e[:, :n_h])
                    ob = spool.tile([P, C_TBL_MAX], F32, tag="ob")
                    nc.vector.tensor_tensor(
                        out=ob[:, :C].rearrange("p (h c) -> p h c", h=n_h),
                        in0=ps_nd[:, :C].rearrange("p (h c) -> p h c", h=n_h),
                        in1=rden[:, :n_h].rearrange("p (h c) -> p h c", c=1)
                            .to_broadcast([P, n_h, c_h]),
                        op=mybir.AluOpType.mult)
                    if not last:
                        # h block -> transposed -> next layer's xl/xr rows
                        hb = spool.tile([P, C_TBL_MAX], BF16, tag="hb")
                        nc.scalar.activation(hb[:, :C], ob[:, :C],
                                             mybir.ActivationFunctionType.Relu)
                        hbT = spool.tile([P, 2 * P], BF16, tag="hbT")
                        for kc in range(2):
                            nc.sync.dma_start(
                                out=hbT[:, kc * P:(kc + 1) * P],
                                in_=hb[:, kc * P:(kc + 1) * P],
                                transpose=True)
                        if l == 0 and debug:
                            nc.sync.dma_start(
                                out=dbg_h1[b * P:(b + 1) * P, :],
                                in_=hb[:, :C])
                            nc.sync.dma_start(
                                out=dbg_h1T[:, b * 2 * P:(b + 1) * 2 * P],
                                in_=hbT[:, :])
                        ps_t = ppool.tile([P, 2 * C_TBL_MAX], F32, tag="ps_t2")
                        for kc in range(2):
                            nc.tensor.matmul(
                                ps_t[:, :2 * Cn],
                                lhsT=hbT[:, kc * P:(kc + 1) * P],
                                rhs=wlr_nxt[:, kc * 2 * Cn:(kc + 1) * 2 * Cn],
                                start=(kc == 0), stop=(kc == 1))
                        xl_t = spool.tile([P, C_TBL_MAX], BF16, tag="xl_t")
                        nc.scalar.activation(xl_t[:, :Cn], ps_t[:, :Cn],
                                             mybir.ActivationFunctionType.Copy)
                        nc.vector.tensor_copy(
                            out=xr_nxt[:, b * Cn:(b + 1) * Cn],
                            in_=ps_t[:, Cn:2 * Cn])
                        nc.sync.dma_start(
                            out=xl_loc[l + 1][b * P:(b + 1) * P, :],
                            in_=xl_t[:, :Cn])
                        # xl AllGather (Shared output allows only a single
                        # writer instruction, so one AG per layer)
                        if b == NBLK - 1:
                            nc.gpsimd.collective_compute(
                                "AllGather", mybir.AluOpType.bypass,
                                replica_groups=rg,
                                ins=[xl_loc[l + 1][:, :].opt()],
                                outs=[xl_tbl[l + 1][:, :].opt()])
                    else:
                        nc.sync.dma_start(
                            out=out_d[b * P:(b + 1) * P, :],
                            in_=ob[:, :OUT_CH])
                if l == 0 and debug:
                    nc.sync.dma_start(out=dbg_xr1[:, :], in_=xr_cur[:, :NBLK * 256])
                if not last:
                    xr_cur = xr_nxt
            if debug:
                nc.sync.dma_start(out=dbg_tbl0[:, :], in_=xl_tbl[0][:, :])
                nc.sync.dma_start(out=dbg_tbl1[:, :], in_=xl_tbl[1][:, :])
                nc.sync.dma_start(out=dbg_loc1[:, :], in_=xl_loc[1][:, :])
    nc.compile()
    return nc


# ---------------------------------------------------------------------------
# host orchestration
# ---------------------------------------------------------------------------

def _wT_pad(w, c_tbl):
    """w: [h*oc, ic] fp32 -> [ic, c_tbl] bf16 (zero pad the out channels)."""
    import ml_dtypes
    w = np.asarray(w, np.float32)
    hoc, ic = w.shape
    out = np.zeros((ic, c_tbl), ml_dtypes.bfloat16)
    out[:, :hoc] = w.T.astype(ml_dtypes.bfloat16)
    return out


def _a_rep(a, c_tbl):
    """a: [h, oc] fp32 -> [128, c_tbl] fp16 replicated across partitions."""
    a = np.asarray(a, np.float32).reshape(-1)
    row = np.zeros(c_tbl, np.float16)
    row[:a.shape[0]] = a.astype(np.float16)
    return np.tile(row[None, :], (P, 1))


def make_in_maps(G, per_core, x, weights):
    import ml_dtypes
    xpad = np.zeros((NPAD, IN_CH), np.float32)
    xpad[:N_REAL] = np.asarray(x, np.float32)
    # full x^T in the chunked row layout (replicated to all cores)
    rows = row_of_node(np.arange(NPAD, dtype=np.int64))
    xT = np.zeros((P, NPAD), ml_dtypes.bfloat16)
    xT[:, rows] = xpad.T.astype(ml_dtypes.bfloat16)
    shared = {"xT": xT}
    for l, L in enumerate(LAYERS):
        wl, wr, a = weights[l]
        shared[f"w{l}l"] = _wT_pad(wl, L["c_tbl"])
        shared[f"w{l}r"] = _wT_pad(wr, L["c_tbl"])
        shared[f"a{l}"] = _a_rep(a, L["c_tbl"])
    in_maps = []
    for c in range(N_CORES):
        m = dict(shared)
        m["xTo"] = np.ascontiguousarray(
            xpad[c * NPC:(c + 1) * NPC].T.astype(ml_dtypes.bfloat16))
        m.update(per_core[c])
        in_maps.append(m)
    return in_maps


_CACHE = {}


def _get_built(edge_index):
    key = hash(np.asarray(edge_index).tobytes())
    if key not in _CACHE:
        G, per_core = prep_graph(edge_index)
        nc = build_nc(G)
        _CACHE[key] = (G, per_core, nc)
    return _CACHE[key]


def kernel(x, edge_index,
           w1l, b1l, w1r, b1r, a1, bo1,
           w2l, b2l, w2r, b2r, a2, bo2,
           w3l, b3l, w3r, b3r, a3, bo3,
           w4l, b4l, w4r, b4r, a4, bo4,
           _trace=False):
    for b in (b1l, b1r, b2l, b2r, b3l, b3r, b4l, b4r, bo1, bo2, bo3):
        assert np.max(np.abs(np.asarray(b, np.float32))) == 0.0, \
            "non-zero internal biases not supported"
    G, per_core, nc = _get_built(edge_index)
    weights = [(w1l, w1r, a1), (w2l, w2r, a2), (w3l, w3r, a3), (w4l, w4r, a4)]
    in_maps = make_in_maps(G, per_core, x, weights)
    res = run_bass_kernel_spmd(nc, in_maps, core_ids=list(range(N_CORES)),
                               trace=_trace)
    outs = [np.asarray(res.results[c]["out"]) for c in range(N_CORES)]
    full = np.concatenate(outs, axis=0)[:N_REAL].astype(np.float32)
    full = full + np.asarray(bo4, np.float32)[None, :]
    if _trace:
        kernel.last_exec_time_ns = res.exec_time_ns
        kernel.last_res = res
    return full


kernel.last_exec_time_ns = None
kernel.last_res = None


# revision 21
# speedup vs baseline: 1.1453x; 1.1453x over previous
"""4-layer GATv2 forward pass on 8 TRN2 NeuronCores (Bass/Tile).

Strategy (node/dst partitioning, pipelined cross-core xl exchange):
  - Nodes are padded to 20480 and split into 8 contiguous slices of 2560
    (20 blocks of 128 dst nodes per core).  Each core owns the segment
    softmax + weighted scatter for its dst nodes, so all softmax
    reductions are core-local.
  - Edges (with self loops appended) are routed to the core/block that
    owns their dst.  Per (core, block) edge counts are padded to a
    shared multiple of 128 (G[b] groups of 128 edges) so one NEFF works
    for all 8 cores.
  - The per-layer xl tables (gather source for xl[src]) are produced
    INSIDE the previous layer's edge phase: as each dst block's h is
    computed, it is DMA-transposed (XBAR) and immediately matmul'ed by
    the next layer's wl/wr.  xl rows stream to a local DRAM buffer that
    is AllGather'ed in 4 chunks overlapping the edge phase, so the
    inter-layer collective is almost entirely hidden.  Layer 1 needs no
    collective at all: x^T is staged as a replicated input and every
    core computes the full layer-1 xl table itself.
  - Per-edge work is edge-major (partition = edge % 128): dma_gather of
    xl[src] rows, z = Sel^T @ xr + I @ xl (PE), leaky-relu (scalar),
    per-head dot with `a` (DVE reduce), exp, then segment softmax
    denominator and the alpha-weighted sum of xl[src] as one PE matmul
    per 128-edge group against a one-hot dst selector (SelT).
  - Softmax uses exp(logit) directly (no running max): logits are O(10)
    here, fp32 exp is exact enough, and the math is identical to the
    reference's shifted softmax.

kernel(**inputs) takes the full problem inputs and returns the full
[20000, 16] fp32 output.
"""

import numpy as np

import concourse.bass as bass
import concourse.bacc as bacc
import concourse.mybir as mybir
import concourse.tile as tile
from concourse.bass_utils import run_bass_kernel_spmd
from concourse.masks import make_identity

F16 = mybir.dt.float16
BF16 = mybir.dt.bfloat16
F32 = mybir.dt.float32
I16 = mybir.dt.int16
P = 128

# model dims (fixed by the problem)
N_REAL = 20000
E_RAW = 320000
IN_CH = 128
HID = 64
HEADS = 4
OUT_CH = 16
SLOPE = 0.2

MASK_NEG = -50.0  # additive logit bias for pad edges
DEN_EPS = 1e-12   # keeps reciprocal() in range for edgeless (pad) dst rows

N_CORES = 8
NPC = 2560          # nodes per core (padded)
NBLK = NPC // P     # 20 dst blocks per core
NPAD = N_CORES * NPC
NCHUNK = 4          # xl AllGather chunks per layer
CBLK = NBLK // NCHUNK           # 5 blocks per chunk
CROWS = CBLK * P                # 640 rows per core per chunk
# emit AllGather for chunk k after this edge block:
AG_EMIT = (11, 14, 17, 19)

# per-layer table widths (c_tbl) and heads
LAYERS = [
    dict(c_in=IN_CH, c_tbl=HEADS * HID, n_h=HEADS, c_h=HID),
    dict(c_in=HEADS * HID, c_tbl=HEADS * HID, n_h=HEADS, c_h=HID),
    dict(c_in=HEADS * HID, c_tbl=HEADS * HID, n_h=HEADS, c_h=HID),
    dict(c_in=HEADS * HID, c_tbl=P, n_h=1, c_h=P),  # 16 real, padded to 128
]
C_TBL_MAX = max(L["c_tbl"] for L in LAYERS)


def row_of_node(n):
    """Chunked xl-table row layout shared by all 4 layers: AllGather chunk k
    (blocks 5k..5k+4 of each core), then core, then block-in-chunk."""
    c, rem = n // NPC, n % NPC
    b, p = rem // P, rem % P
    k, j = b // CBLK, b % CBLK
    return ((k * N_CORES + c) * CBLK + j) * P + p


# ---------------------------------------------------------------------------
# host-side graph preprocessing
# ---------------------------------------------------------------------------

def prep_graph(edge_index):
    """Route edges (plus self loops) to (core, block) by dst; build per-core
    gather-index / dst-local / mask arrays in the exact SBUF layouts the
    kernel consumes."""
    n = N_REAL
    src = np.concatenate([np.asarray(edge_index[0], np.int64),
                          np.arange(n, dtype=np.int64)])
    dst = np.concatenate([np.asarray(edge_index[1], np.int64),
                          np.arange(n, dtype=np.int64)])
    assert src.min() >= 0 and src.max() < n and dst.min() >= 0 and dst.max() < n

    gblk = dst // P                       # global block id (core-major)
    order = np.argsort(gblk, kind="stable")
    src, dst, gblk = src[order], dst[order], gblk[order]

    nblk_tot = N_CORES * NBLK
    counts = np.bincount(gblk, minlength=nblk_tot).reshape(N_CORES, NBLK)
    G = np.maximum(1, (counts.max(axis=0) + P - 1) // P).astype(np.int64)  # [NBLK]
    W = int(G.sum())

    # split edges per (core, block)
    starts = np.zeros(nblk_tot + 1, np.int64)
    np.cumsum(counts.reshape(-1), out=starts[1:])

    # node id -> chunked xl-table row (same for every layer)
    rows = row_of_node(np.arange(n, dtype=np.int64))

    import ml_dtypes
    per_core = []
    for c in range(N_CORES):
        xl_idx = np.zeros((P, 8 * W), np.int16)
        mbias = np.full((P, W), MASK_NEG, np.float16)
        # per block: [sel (gG*128) | selt (gG*128)] back to back
        selcat = np.zeros((P, 2 * W * P), ml_dtypes.bfloat16)
        off = 0
        for b in range(NBLK):
            gb = c * NBLK + b
            s, e = starts[gb], starts[gb + 1]
            nreal = int(e - s)
            g = int(G[b])
            npad_e = g * P
            fsrc = np.zeros(npad_e, np.int64)
            fdl = np.zeros(npad_e, np.int64)
            fm = np.full(npad_e, MASK_NEG, np.float32)
            fsrc[:nreal] = rows[src[s:e]]
            fdl[:nreal] = dst[s:e] % P
            fm[:nreal] = 0.0
            # edge i -> partition i % 128, group i // 128
            mbias[:, off:off + g] = fm.reshape(g, P).T.astype(np.float16)
            fdl_blk = fdl.reshape(g, P)
            co = 2 * off * P
            # Sel[d, g*128+j] = (fdl[g*128+j] == d)  [xr expansion matmul lhsT]
            selcat[:, co:co + g * P] = (
                fdl_blk[None, :, :] == np.arange(P)[:, None, None]
            ).reshape(P, g * P).astype(ml_dtypes.bfloat16)
            # SelT[p, g*128+d] = (fdl[g*128+p] == d)  [segment-sum matmul lhsT]
            selcat[:, co + g * P:co + 2 * g * P] = (
                fdl_blk[:, :, None] == np.arange(P)[None, None, :]
            ).transpose(1, 0, 2).reshape(P, g * P).astype(ml_dtypes.bfloat16)
            # wrapped idx layout: wrapped[p, s] = flat[s*16 + p], replicated
            # into all 8 16-partition groups (one per GPSIMD Q7 core)
            xl_idx[:, 8 * off:8 * (off + g)] = np.tile(
                fsrc.astype(np.int16).reshape(-1, 16).T, (8, 1))
            off += g
        per_core.append(dict(xl_idx=xl_idx, mbias=mbias, selcat=selcat))
    return [int(g) for g in G], per_core


# ---------------------------------------------------------------------------
# bass program
# ---------------------------------------------------------------------------

def build_nc(G, debug=False):
    """Build the (single, SPMD) bass program."""
    nl = len(LAYERS)
    W = sum(G)
    Gmax = max(G)

    nc = bacc.Bacc("TRN2", target_bir_lowering=False, debug=False,
                   num_devices=N_CORES, num_swdge_queues=4)

    # replicated full x^T in the chunked row layout; own-slice x^T for xr1
    xT_d = nc.dram_tensor("xT", [P, NPAD], BF16, kind="ExternalInput")
    xTo_d = nc.dram_tensor("xTo", [P, NPC], BF16, kind="ExternalInput")
    xl_idx_d = nc.dram_tensor("xl_idx", [P, 8 * W], I16, kind="ExternalInput")
    selcat_d = nc.dram_tensor("selcat", [P, 2 * W * P], BF16,
                              kind="ExternalInput")
    mbias_d = nc.dram_tensor("mbias", [P, W], F16, kind="ExternalInput")
    w_d, a_d = [], []
    for l, L in enumerate(LAYERS):
        wl = nc.dram_tensor(f"w{l}l", [L["c_in"], L["c_tbl"]], BF16,
                            kind="ExternalInput")
        wr = nc.dram_tensor(f"w{l}r", [L["c_in"], L["c_tbl"]], BF16,
                            kind="ExternalInput")
        w_d.append((wl, wr))
        a_d.append(nc.dram_tensor(f"a{l}", [P, L["c_tbl"]], F16,
                                  kind="ExternalInput"))
    out_d = nc.dram_tensor("out", [NPC, OUT_CH], F32, kind="ExternalOutput")
    if debug:
        dbg_tbl0 = nc.dram_tensor("dbg_tbl0", [NPAD, 256], BF16,
                                  kind="ExternalOutput")
        dbg_tbl1 = nc.dram_tensor("dbg_tbl1", [NPAD, 256], BF16,
                                  kind="ExternalOutput")
        dbg_loc1 = nc.dram_tensor("dbg_loc1", [NPC, 256], BF16,
                                  kind="ExternalOutput")
        dbg_xr1 = nc.dram_tensor("dbg_xr1", [P, NBLK * 256], BF16,
                                 kind="ExternalOutput")
        dbg_h1 = nc.dram_tensor("dbg_h1", [NPC, 256], BF16,
                                kind="ExternalOutput")
        dbg_h1T = nc.dram_tensor("dbg_h1T", [P, NBLK * 256], BF16,
                                 kind="ExternalOutput")
        dbg_xlg = nc.dram_tensor("dbg_xlg", [P, 18 * 256], BF16,
                                 kind="ExternalOutput")
        dbg_lrz = nc.dram_tensor("dbg_lrz", [P, 18 * 256], F16,
                                 kind="ExternalOutput")
        dbg_ex = nc.dram_tensor("dbg_ex", [P, 18 * HEADS], BF16,
                                kind="ExternalOutput")

    rg = [list(range(N_CORES))]

    with tile.TileContext(nc) as tc:
        with (
            tc.tile_pool(name="const", bufs=1) as cpool,
            tc.tile_pool(name="wts", bufs=2) as wpool,
            tc.tile_pool(name="mm", bufs=4) as mpool,
            tc.tile_pool(name="gath", bufs=2) as gpool,
            tc.tile_pool(name="edge", bufs=2) as epool,
            tc.tile_pool(name="small", bufs=2) as spool,
            tc.tile_pool(name="psum", bufs=2, space="PSUM") as ppool,
            tc.tile_pool(name="dram", bufs=1, space="DRAM") as dpool,
        ):
            # ---- persistent constants -------------------------------------
            ident = cpool.tile([P, P], BF16, tag="ident")
            make_identity(nc, ident[:])
            mb_sb = cpool.tile([P, W], F16, tag="mbias")
            nc.sync.dma_start(out=mb_sb[:], in_=mbias_d[:])

            # ---- per-layer DRAM scratch -----------------------------------
            # xl gather tables (chunked row layout). L1 local (computed
            # replicated), L2-4 shared (AllGather outputs).
            # Local output (not Shared): the sim allows only a single writer
            # instruction for Shared tensors, and we AllGather in 4 chunks.
            xl_tbl = []
            xl_loc = [None]
            for l, L in enumerate(LAYERS):
                xl_tbl.append(dpool.tile(
                    [NPAD, L["c_tbl"]], BF16, tag=f"xltbl{l}",
                    name=f"xltbl{l}"))
                if l > 0:
                    xl_loc.append(dpool.tile([NPC, L["c_tbl"]], BF16,
                                             tag=f"xlloc{l}", name=f"xlloc{l}"))

            # ---- L1 node phase (replicated; no collective) ----------------
            wl_sb = wpool.tile([P, 2 * C_TBL_MAX], BF16, tag="wl")
            wr_sb = wpool.tile([P, 2 * C_TBL_MAX], BF16, tag="wr")
            C1 = LAYERS[0]["c_tbl"]
            nc.sync.dma_start(out=wl_sb[:, :C1], in_=w_d[0][0][:, :])
            nc.sync.dma_start(out=wr_sb[:, :C1], in_=w_d[0][1][:, :])
            for rb4 in range(0, NPAD // P, 4):
                xt4 = mpool.tile([P, 4 * P], BF16, tag="xt4")
                nc.sync.dma_start(out=xt4[:],
                                  in_=xT_d[:, rb4 * P:(rb4 + 4) * P])
                xlq = mpool.tile([P, 4 * C1], BF16, tag="xlq")
                for q in range(4):
                    ps1 = ppool.tile([P, 2 * C_TBL_MAX], F32, tag="ps_t2")
                    nc.tensor.matmul(ps1[:, :C1],
                                     lhsT=xt4[:, q * P:(q + 1) * P],
                                     rhs=wl_sb[:, :C1], start=True, stop=True)
                    if q % 2 == 0:
                        nc.scalar.activation(xlq[:, q * C1:(q + 1) * C1],
                                             ps1[:, :C1],
                                             mybir.ActivationFunctionType.Copy)
                    else:
                        nc.vector.tensor_copy(out=xlq[:, q * C1:(q + 1) * C1],
                                              in_=ps1[:, :C1])
                nc.sync.dma_start(
                    out=xl_tbl[0][rb4 * P:(rb4 + 4) * P, :]
                        .rearrange("(q p) c -> p q c", q=4),
                    in_=xlq[:].rearrange("p (q c) -> p q c", q=4))
            xr_cur = wpool.tile([P, NBLK * C_TBL_MAX], BF16, tag="xr_sb")
            for b in range(NBLK):
                xto = mpool.tile([P, P], BF16, tag="xt")
                nc.sync.dma_start(out=xto[:], in_=xTo_d[:, b * P:(b + 1) * P])
                ps1 = ppool.tile([P, 2 * C_TBL_MAX], F32, tag="ps_t2")
                nc.tensor.matmul(ps1[:, :C1], lhsT=xto[:], rhs=wr_sb[:, :C1],
                                 start=True, stop=True)
                if b % 2 == 0:
                    nc.scalar.activation(xr_cur[:, b * C1:(b + 1) * C1],
                                         ps1[:, :C1],
                                         mybir.ActivationFunctionType.Copy)
                else:
                    nc.vector.tensor_copy(out=xr_cur[:, b * C1:(b + 1) * C1],
                                          in_=ps1[:, :C1])

            # ---- layers ----------------------------------------------------
            for l, L in enumerate(LAYERS):
                C, n_h, c_h = L["c_tbl"], L["n_h"], L["c_h"]
                EC = C + n_h
                last = l + 1 == nl
                if not last:
                    Cn = LAYERS[l + 1]["c_tbl"]
                    # next layer's weights, packed [wl_kc | wr_kc] per kc so
                    # the tail is one accumulation chain per PSUM bank
                    wlr_nxt = wpool.tile([P, 4 * C_TBL_MAX], BF16, tag="wlr")
                    for kc in range(2):
                        nc.sync.dma_start(
                            out=wlr_nxt[:, kc * 2 * Cn:kc * 2 * Cn + Cn],
                            in_=w_d[l + 1][0][kc * P:(kc + 1) * P, :])
                        nc.sync.dma_start(
                            out=wlr_nxt[:, kc * 2 * Cn + Cn:(kc + 1) * 2 * Cn],
                            in_=w_d[l + 1][1][kc * P:(kc + 1) * P, :])
                    xr_nxt = wpool.tile([P, NBLK * C_TBL_MAX], BF16,
                                        tag="xr_sb")
                a_rep = wpool.tile([P, Gmax * C_TBL_MAX], F16, tag="arep")
                nc.sync.dma_start(
                    out=a_rep[:, :Gmax * C].rearrange("p (g c) -> p g c",
                                                      g=Gmax),
                    in_=a_d[l][:].rearrange("p (g c) -> p g c", g=1)
                        .to_broadcast([P, Gmax, C]))

                # ---- edge phase -------------------------------------------
                qn = [0]

                def gather_rows(tbl_ap, out_tile, off_g, n_g, C_, nm):
                    """Gather n_g*128 rows from tbl_ap into out_tile
                    [P, n_g, C_], in <=1024-idx chunks (ucode ring limit)
                    round-robined over the 4 SWDGE queues."""
                    for k0 in range(0, n_g, 8):
                        gk = min(8, n_g - k0)
                        it = gpool.tile([P, 8 * 8], I16, tag="idxt",
                                        name=f"idxt_{nm}_{k0}", bufs=8)
                        nc.sync.dma_start(
                            out=it[:, :8 * gk],
                            in_=xl_idx_d[:, 8 * (off_g + k0):
                                         8 * (off_g + k0 + gk)])
                        nc.gpsimd.dma_gather(
                            out_ap=out_tile[:, k0 * C_:(k0 + gk) * C_]
                                .rearrange("p (g c) -> p g c", c=C_),
                            in_ap=tbl_ap,
                            idxs_ap=it[:, :8 * gk],
                            num_idxs=gk * P, num_idxs_reg=gk * P,
                            elem_size=C_, queue_num=qn[0] % 4)
                        qn[0] += 1

                for b in range(NBLK):
                    gG = G[b]
                    off = sum(G[:b])
                    xl_g = gpool.tile([P, Gmax * C_TBL_MAX], BF16, tag="xl_g",
                                      bufs=4)
                    gather_rows(xl_tbl[l][:, :], xl_g, off, gG, C,
                                f"xl{l}_{b}")
                    xlb = xl_g[:, :gG * C]
                    # [sel | selt] for this block in one DMA
                    selcat = epool.tile([P, 2 * Gmax * P], BF16, tag="selcat")
                    nc.sync.dma_start(
                        out=selcat[:, :2 * gG * P],
                        in_=selcat_d[:, 2 * off * P:2 * (off + gG) * P])
                    sel = selcat[:, :gG * P]          # [d, e]-major
                    selt = selcat[:, gG * P:2 * gG * P]  # [e, d]-major
                    # z (per pair of groups) in PSUM:
                    #   z_g = Sel_g^T @ xr_blk + I^T @ xl_g   -> leaky relu
                    # NOTE: start=True wipes the whole PSUM bank, so each
                    # bank's accumulation must begin with ONE start=True
                    # matmul covering every region used (the ident batch),
                    # then accumulate into sub-regions with start=False.
                    lrz = epool.tile([P, Gmax * C_TBL_MAX], F16, tag="lrz")
                    for g0 in range(0, gG, 2):
                        gns = min(2, gG - g0)
                        ps_z = ppool.tile([P, 2 * C_TBL_MAX], F32, tag="ps_z")
                        nc.tensor.matmul(
                            ps_z[:, :gns * C], lhsT=ident[:],
                            rhs=xl_g[:, g0 * C:(g0 + gns) * C],
                            start=True, stop=False)
                        for gg in range(g0, g0 + gns):
                            sl = slice((gg - g0) * C, (gg - g0 + 1) * C)
                            nc.tensor.matmul(
                                ps_z[:, sl], lhsT=sel[:, gg * P:(gg + 1) * P],
                                rhs=xr_cur[:, b * C:(b + 1) * C],
                                start=False, stop=(gg == g0 + gns - 1))
                        nc.scalar.activation(
                            lrz[:, g0 * C:(g0 + gns) * C],
                            ps_z[:, :gns * C],
                            mybir.ActivationFunctionType.Prelu,
                            alpha=SLOPE)
                    # a * LR(z)
                    alr = epool.tile([P, Gmax * C_TBL_MAX], F16, tag="alr")
                    nc.vector.tensor_tensor(out=alr[:, :gG * C],
                                            in0=lrz[:, :gG * C],
                                            in1=a_rep[:, :gG * C],
                                            op=mybir.AluOpType.mult)
                    # logits: two folds + reduce over c_h/4 (faster on DVE
                    # than one strided reduce)
                    ch2, ch4 = c_h // 2, c_h // 4
                    fold1 = spool.tile([P, Gmax * C_TBL_MAX // 2], F16,
                                       tag="fold1")
                    a4 = alr[:, :gG * C].rearrange(
                        "p (g h c) -> p g h c", h=n_h, c=c_h)
                    f13 = fold1[:, :gG * C // 2].rearrange(
                        "p (g h c) -> p g h c", h=n_h, c=ch2)
                    nc.vector.tensor_tensor(out=f13, in0=a4[:, :, :, :ch2],
                                            in1=a4[:, :, :, ch2:],
                                            op=mybir.AluOpType.add)
                    fold2 = spool.tile([P, Gmax * C_TBL_MAX // 4], F16,
                                       tag="fold2")
                    f23 = fold2[:, :gG * C // 4].rearrange(
                        "p (g h c) -> p g h c", h=n_h, c=ch4)
                    nc.vector.tensor_tensor(out=f23, in0=f13[:, :, :, :ch4],
                                            in1=f13[:, :, :, ch4:],
                                            op=mybir.AluOpType.add)
                    logits = spool.tile([P, Gmax * HEADS], F32, tag="logits")
                    nc.vector.tensor_reduce(
                        out=logits[:, :gG * n_h].rearrange(
                            "p (g h) -> p g h", h=n_h),
                        in_=f23,
                        axis=mybir.AxisListType.X, op=mybir.AluOpType.add)
                    # pad-edge mask as additive bias
                    logm = spool.tile([P, Gmax * HEADS], F32, tag="logm")
                    nc.vector.tensor_tensor(
                        out=logm[:, :gG * n_h].rearrange(
                            "p (g h) -> p g h", h=n_h),
                        in0=logits[:, :gG * n_h].rearrange(
                            "p (g h) -> p g h", h=n_h),
                        in1=mb_sb[:, off:off + gG]
                            .rearrange("p (g h) -> p g h", h=1)
                            .to_broadcast([P, gG, n_h]),
                        op=mybir.AluOpType.add)
                    ex = spool.tile([P, Gmax * HEADS], BF16, tag="ex")
                    nc.scalar.activation(ex[:, :gG * n_h], logm[:, :gG * n_h],
                                         mybir.ActivationFunctionType.Exp)
                    if debug and l == 0 and b == 0:
                        nc.sync.dma_start(out=dbg_xlg[:, :gG * C],
                                          in_=xl_g[:, :gG * C])
                        nc.sync.dma_start(out=dbg_lrz[:, :gG * C],
                                          in_=lrz[:, :gG * C])
                        nc.sync.dma_start(out=dbg_ex[:, :gG * n_h],
                                          in_=ex[:, :gG * n_h])
                    # expand ex over c_h (scalar engine broadcast copy; DVE
                    # broadcast APs are pathologically slow)
                    ex_e = epool.tile([P, Gmax * C_TBL_MAX], BF16, tag="ex_e")
                    nc.scalar.activation(
                        ex_e[:, :gG * C].rearrange(
                            "p (g h c) -> p g h c", h=n_h, c=c_h),
                        ex[:, :gG * n_h].rearrange(
                            "p (g h c) -> p g h c", h=n_h, c=1)
                            .to_broadcast([P, gG, n_h, c_h]),
                        mybir.ActivationFunctionType.Copy)
                    # edata = [ex * xl[src] | ex]
                    edata = epool.tile([P, Gmax * (C_TBL_MAX + HEADS)], BF16,
                                       tag="edata")
                    ed3 = edata[:, :gG * EC].rearrange("p (g c) -> p g c", c=EC)
                    nc.vector.tensor_tensor(
                        out=ed3[:, :, :C],
                        in0=xlb.rearrange("p (g c) -> p g c", c=C),
                        in1=ex_e[:, :gG * C].rearrange("p (g c) -> p g c",
                                                       c=C),
                        op=mybir.AluOpType.mult)
                    nc.vector.tensor_copy(
                        out=ed3[:, :, C:],
                        in_=ex[:, :gG * n_h].rearrange("p (g h) -> p g h",
                                                       h=n_h))
                    # segment sums via PE: psum[d, :] += SelT_g^T @ edata_g
                    ps_nd = ppool.tile([P, EC], F32, tag="ps_nd")
                    for g in range(gG):
                        nc.tensor.matmul(
                            ps_nd[:], lhsT=selt[:, g * P:(g + 1) * P],
                            rhs=edata[:, g * EC:(g + 1) * EC],
                            start=(g == 0), stop=(g == gG - 1))
                    den_e = spool.tile([P, HEADS], F32, tag="den_e")
                    nc.vector.tensor_scalar(
                        out=den_e[:, :n_h], in0=ps_nd[:, C:], scalar1=DEN_EPS,
                        scalar2=None, op0=mybir.AluOpType.add)
                    rden = spool.tile([P, HEADS], F32, tag="rden")
                    nc.vector.reciprocal(rden[:, :n_h], den_e[:, :n_h])
                    ob = spool.tile([P, C_TBL_MAX], F32, tag="ob")
                    nc.vector.tensor_tensor(
                        out=ob[:, :C].rearrange("p (h c) -> p h c", h=n_h),
                        in0=ps_nd[:, :C].rearrange("p (h c) -> p h c", h=n_h),
                        in1=rden[:, :n_h].rearrange("p (h c) -> p h c", c=1)
                            .to_broadcast([P, n_h, c_h]),
                        op=mybir.AluOpType.mult)
                    if not last:
                        # h block -> transposed -> next layer's xl/xr rows
                        hb = spool.tile([P, C_TBL_MAX], BF16, tag="hb")
                        nc.scalar.activation(hb[:, :C], ob[:, :C],
                                             mybir.ActivationFunctionType.Relu)
                        hbT = spool.tile([P, 2 * P], BF16, tag="hbT")
                        for kc in range(2):
                            nc.sync.dma_start(
                                out=hbT[:, kc * P:(kc + 1) * P],
                                in_=hb[:, kc * P:(kc + 1) * P],
                                transpose=True)
                        if l == 0 and debug:
                            nc.sync.dma_start(
                                out=dbg_h1[b * P:(b + 1) * P, :],
                                in_=hb[:, :C])
                            nc.sync.dma_start(
                                out=dbg_h1T[:, b * 2 * P:(b + 1) * 2 * P],
                                in_=hbT[:, :])
                        ps_t = ppool.tile([P, 2 * C_TBL_MAX], F32, tag="ps_t2")
                        for kc in range(2):
                            nc.tensor.matmul(
                                ps_t[:, :2 * Cn],
                                lhsT=hbT[:, kc * P:(kc + 1) * P],
                                rhs=wlr_nxt[:, kc * 2 * Cn:(kc + 1) * 2 * Cn],
                                start=(kc == 0), stop=(kc == 1))
                        xl_t = spool.tile([P, C_TBL_MAX], BF16, tag="xl_t")
                        nc.scalar.activation(xl_t[:, :Cn], ps_t[:, :Cn],
                                             mybir.ActivationFunctionType.Copy)
                        nc.vector.tensor_copy(
                            out=xr_nxt[:, b * Cn:(b + 1) * Cn],
                            in_=ps_t[:, Cn:2 * Cn])
                        nc.sync.dma_start(
                            out=xl_loc[l + 1][b * P:(b + 1) * P, :],
                            in_=xl_t[:, :Cn])
                        # pipelined xl AllGather chunks (local output so
                        # multiple writer instructions are allowed)
                        if b in AG_EMIT:
                            k = AG_EMIT.index(b)
                            nc.gpsimd.collective_compute(
                                "AllGather", mybir.AluOpType.bypass,
                                replica_groups=rg,
                                ins=[xl_loc[l + 1][k * CROWS:(k + 1) * CROWS,
                                                   :].opt()],
                                outs=[xl_tbl[l + 1][k * N_CORES * CROWS:
                                                    (k + 1) * N_CORES * CROWS,
                                                    :].opt()])
                    else:
                        nc.sync.dma_start(
                            out=out_d[b * P:(b + 1) * P, :],
                            in_=ob[:, :OUT_CH])
                if l == 0 and debug:
                    nc.sync.dma_start(out=dbg_xr1[:, :], in_=xr_cur[:, :NBLK * 256])
                if not last:
                    xr_cur = xr_nxt
            if debug:
                nc.sync.dma_start(out=dbg_tbl0[:, :], in_=xl_tbl[0][:, :])
                nc.sync.dma_start(out=dbg_tbl1[:, :], in_=xl_tbl[1][:, :])
                nc.sync.dma_start(out=dbg_loc1[:, :], in_=xl_loc[1][:, :])
    nc.compile()
    return nc


# ---------------------------------------------------------------------------
# host orchestration
# ---------------------------------------------------------------------------

def _wT_pad(w, c_tbl):
    """w: [h*oc, ic] fp32 -> [ic, c_tbl] bf16 (zero pad the out channels)."""
    import ml_dtypes
    w = np.asarray(w, np.float32)
    hoc, ic = w.shape
    out = np.zeros((ic, c_tbl), ml_dtypes.bfloat16)
    out[:, :hoc] = w.T.astype(ml_dtypes.bfloat16)
    return out


def _a_rep(a, c_tbl):
    """a: [h, oc] fp32 -> [128, c_tbl] fp16 replicated across partitions."""
    a = np.asarray(a, np.float32).reshape(-1)
    row = np.zeros(c_tbl, np.float16)
    row[:a.shape[0]] = a.astype(np.float16)
    return np.tile(row[None, :], (P, 1))


def make_in_maps(G, per_core, x, weights):
    import ml_dtypes
    xpad = np.zeros((NPAD, IN_CH), np.float32)
    xpad[:N_REAL] = np.asarray(x, np.float32)
    # full x^T in the chunked row layout (replicated to all cores)
    rows = row_of_node(np.arange(NPAD, dtype=np.int64))
    xT = np.zeros((P, NPAD), ml_dtypes.bfloat16)
    xT[:, rows] = xpad.T.astype(ml_dtypes.bfloat16)
    shared = {"xT": xT}
    for l, L in enumerate(LAYERS):
        wl, wr, a = weights[l]
        shared[f"w{l}l"] = _wT_pad(wl, L["c_tbl"])
        shared[f"w{l}r"] = _wT_pad(wr, L["c_tbl"])
        shared[f"a{l}"] = _a_rep(a, L["c_tbl"])
    in_maps = []
    for c in range(N_CORES):
        m = dict(shared)
        m["xTo"] = np.ascontiguousarray(
            xpad[c * NPC:(c + 1) * NPC].T.astype(ml_dtypes.bfloat16))
        m.update(per_core[c])
        in_maps.append(m)
    return in_maps


_CACHE = {}


def _get_built(edge_index):
    key = hash(np.asarray(edge_index).tobytes())
    if key not in _CACHE:
        G, per_core = prep_graph(edge_index)
        nc = build_nc(G)
        _CACHE[key] = (G, per_core, nc)
    return _CACHE[key]


def kernel(x, edge_index,
           w1l, b1l, w1r, b1r, a1, bo1,
           w2l, b2l, w2r, b2r, a2, bo2,
           w3l, b3l, w3r, b3r, a3, bo3,
           w4l, b4l, w4r, b4r, a4, bo4,
           _trace=False):
    for b in (b1l, b1r, b2l, b2r, b3l, b3r, b4l, b4r, bo1, bo2, bo3):
        assert np.max(np.abs(np.asarray(b, np.float32))) == 0.0, \
            "non-zero internal biases not supported"
    G, per_core, nc = _get_built(edge_index)
    weights = [(w1l, w1r, a1), (w2l, w2r, a2), (w3l, w3r, a3), (w4l, w4r, a4)]
    in_maps = make_in_maps(G, per_core, x, weights)
    res = run_bass_kernel_spmd(nc, in_maps, core_ids=list(range(N_CORES)),
                               trace=_trace)
    outs = [np.asarray(res.results[c]["out"]) for c in range(N_CORES)]
    full = np.concatenate(outs, axis=0)[:N_REAL].astype(np.float32)
    full = full + np.asarray(bo4, np.float32)[None, :]
    if _trace:
        kernel.last_exec_time_ns = res.exec_time_ns
        kernel.last_res = res
    return full


kernel.last_exec_time_ns = None
kernel.last_res = None
